# revision 4
# baseline (speedup 1.0000x reference)
"""Grouped-Query Attention on 8 Trainium2 NeuronCores.

Sharding: core c handles (batch b = c//4, query-head group g = c%4).
Each core computes its group's Q projection (256 cols of W_Q), the
group-shared K/V projections, 4 heads of attention over the full
sequence, and a partial output projection against the group's 256 rows
of W_O. The host sums the 4 group partials per batch and adds b_O.

v2 design (all matmuls bf16 operands, fp32 PSUM accumulate):
  - host pre-casts x and all weights to bf16 in SBUF-ready layouts
  - xT produced by hardware DMA-transpose (DRAM -> SBUF), no PE/DVE cost
  - K and V projections fused: stationary [128, 64K|64V] -> KT rows 0-63,
    VT rows 64-127 of one PSUM tile; V natural recovered from VT by a
    second DMA-transpose
  - softmax exp split between ScalarE (exact Exp activation) and DVE
    (Schraudolph bit-trick: PT_bits = int16(score * A + B), one
    tensor_scalar per offloaded chunk) to balance engine busy time
  - ctx matmuls accumulate [V | 1] so col 64 carries the softmax denom;
    evac divides by it (DVE reciprocal + scale-multiply)
  - ctxT produced by DMA-transpose; output projection accumulates over
    the two 128-row q-chunks; DVE evacuates PSUM -> staging -> DMA out

b_V and b_O are applied on the host: b_V adds exactly
(tile(b_V) @ W_O_g) to every output row (softmax weights sum to 1).
"""

import numpy as np

S = 2048
DM = 1024
G = 4
H = 4  # heads per group
DK = 64
GQ = 256  # query width per group
B = 2
NK = DM // 128  # 8 contraction chunks
NT = S // 128  # 16 token chunks
SBLK = 512
NSB = S // SBLK  # 4 query super-blocks

# Schraudolph constants: exp(s/8) = 2^(s*0.125*log2 e); bf16 bits via
# int16(s*A + B). B includes the -5.5 minimax offset and +0.5 for the
# truncating float->int convert on hardware.
EXP_A = float(np.float32(128.0 * 0.125 * np.log2(np.e)))
EXP_B = float(np.float32(127.0 * 128.0 - 5.5 + 0.5))
# t-chunks (of 16) whose exp goes to DVE instead of ScalarE
OFF_T = (1, 4, 7, 10, 13)

_CACHED = {}


def _split_sync_waits(nc, drain_max=1, other_max=1):
    """This walrus build has a single sync-wait slot on CTRL-class
    instructions (Drain/NoOp); Tile's exit drain collects 3+. Move the
    excess onto preceding single-wait NOPs on the same engine."""
    import concourse.mybir as mybir
    import bass_rust

    n_split = 0
    for f in nc.m.functions:
        for bb in f.blocks:
            out = []
            changed = False
            for inst in bb.instructions:
                si = getattr(inst, "sync_info", None)
                limit = drain_max if type(inst).__name__ in ("InstDrain", "InstNoOp") else other_max
                if si is not None and len(si.on_wait) > limit:
                    waits = list(si.on_wait)
                    keep = waits[-limit:] if limit else []
                    head = waits[: len(waits) - limit]
                    for w in head:
                        out.append(
                            mybir.InstNoOp(
                                name=f"{inst.name}-wsp{n_split}",
                                engine=inst.engine,
                                sync_info=mybir.SyncInfo(on_wait=[w], on_update=[]),
                                bass_nofuse=True,
                            )
                        )
                        n_split += 1
                    inst.sync_info = bass_rust.SyncInfo(on_wait=keep, on_update=si.on_update)
                    changed = True
                out.append(inst)
            if changed:
                bb.instructions = out
    return n_split


def _build_nc(iters=1):
    import concourse.bass as bass
    import concourse.mybir as mybir
    import concourse.tile as tile

    F32 = mybir.dt.float32
    BF = mybir.dt.bfloat16
    I16 = mybir.dt.int16

    nc = bass.Bass("TRN2", target_bir_lowering=False, debug=False, num_devices=8)
    x = nc.dram_tensor("x", [S, DM], BF, kind="ExternalInput")
    wq = nc.dram_tensor("wq", [128, NK * 128 * 2], BF, kind="ExternalInput")
    wkv = nc.dram_tensor("wkv", [128, NK * 128], BF, kind="ExternalInput")
    wo = nc.dram_tensor("wo", [128, 2 * DM], BF, kind="ExternalInput")
    bq = nc.dram_tensor("bq", [128, 2], F32, kind="ExternalInput")
    bk = nc.dram_tensor("bk", [64, 1], F32, kind="ExternalInput")
    out = nc.dram_tensor("out", [S, DM], F32, kind="ExternalOutput")

    with tile.TileContext(nc) as tc:
        with (
            tc.tile_pool(name="wts", bufs=1) as wts,
            tc.tile_pool(name="acts", bufs=1) as acts,
            tc.tile_pool(name="sml", bufs=2) as sml,
            tc.tile_pool(name="outp", bufs=4) as outp,
            tc.tile_pool(name="ps_sc", bufs=2, space="PSUM") as ps_sc,
            tc.tile_pool(name="ps_pr", bufs=2, space="PSUM") as ps_pr,
            tc.tile_pool(name="ps_ctx", bufs=2, space="PSUM") as ps_ctx,
        ):
            def _pipeline():
                # ---- constants + weights: direct bf16 loads, no staging ----
                bq_t = wts.tile([128, 2], F32)
                nc.sync.dma_start(bq_t[:], bq[:])
                bk_t = wts.tile([64, 1], F32)
                nc.sync.dma_start(bk_t[:], bk[:])

                wq_bf = wts.tile([128, NK * 256], BF)  # k-chunk k, m: cols k*256+m*128
                nc.sync.dma_start(wq_bf[:], wq[:])
                kv_bf = wts.tile([128, NK * 128], BF)  # chunk k: cols [k*128,+64)=K, +64..128=V
                nc.sync.dma_start(kv_bf[:], wkv[:])
                wo_bf = wts.tile([128, 2 * DM], BF)  # q-chunk cj at cols [cj*DM, ...)
                nc.sync.dma_start(wo_bf[:], wo[:])

                # ---- xT via DMA transpose: xT[p, k, s] = x[s, k*128+p] ----
                xT = acts.tile([128, NK * S], BF)
                xT3 = xT[:].rearrange("p (k s) -> p k s", k=NK)
                for sg in range(NSB):
                    nc.sync.dma_start_transpose(
                        xT3[:, :, sg * SBLK : (sg + 1) * SBLK],
                        x[sg * SBLK : (sg + 1) * SBLK, :],
                    )

                # ---- fused K|V projection ----
                KT = acts.tile([128, S], BF)  # rows 64-127 duplicate 0-63
                VT = acts.tile([64, S], BF)
                Vb = acts.tile([128, NT * (DK + 1)], BF)  # [V | 1] per token chunk

                for sg in range(NSB):
                    ps = ps_pr.tile([128, SBLK], F32, tag="p")
                    for k in range(NK):
                        nc.tensor.matmul(
                            ps[:],
                            kv_bf[:, k * 128 : (k + 1) * 128],
                            xT[:, k * S + sg * SBLK : k * S + (sg + 1) * SBLK],
                            start=(k == 0),
                            stop=(k == NK - 1),
                        )
                    nc.vector.tensor_scalar_add(
                        KT[:64, sg * SBLK : (sg + 1) * SBLK], ps[:64, :], bk_t[:]
                    )
                    nc.vector.tensor_copy(VT[:, sg * SBLK : (sg + 1) * SBLK], ps[64:128, :])
                nc.sync.dma_start(KT[64:128, :], KT[:64, :])
                # V natural from VT by DMA transpose: Vtmp[p, t, d] = VT[d, t*128+p].
                # The transpose needs a contiguous destination (a sliced dst
                # mis-executes on hardware); copy into the 65-col Vb layout after.
                Vtmp = acts.tile([128, NT * DK], BF)
                nc.sync.dma_start_transpose(
                    Vtmp[:].rearrange("p (t c) -> p t c", c=DK), VT[:]
                )
                nc.vector.tensor_copy(
                    Vb[:].rearrange("p (t c) -> p t c", c=DK + 1)[:, :, :DK],
                    Vtmp[:].rearrange("p (t c) -> p t c", c=DK),
                )
                nc.gpsimd.memset(Vb[:].rearrange("p (t c) -> p t c", c=DK + 1)[:, :, DK], 1.0)

                # ---- Q projection (m-tile m holds heads 2m, 2m+1) ----
                QT = acts.tile([128, 2 * S], BF)

                def q_proj(m, sg):
                    ps = ps_pr.tile([128, SBLK], F32, tag="p")
                    for k in range(NK):
                        nc.tensor.matmul(
                            ps[:],
                            wq_bf[:, k * 256 + m * 128 : k * 256 + (m + 1) * 128],
                            xT[:, k * S + sg * SBLK : k * S + (sg + 1) * SBLK],
                            start=(k == 0),
                            stop=(k == NK - 1),
                        )
                    nc.vector.tensor_scalar_add(
                        QT[:, m * S + sg * SBLK : m * S + (sg + 1) * SBLK], ps[:], bq_t[:, m : m + 1]
                    )

                for m in range(2):
                    q_proj(m, 0)

                # ---- attention + output, per query super-block ----
                # PT split into p-halves so sb+1 scores can reuse the first
                # half while ctx of heads 2,3 still reads the second.
                PT = [acts.tile([128, NT * 2 * SBLK], BF, name=f"pt{p}") for p in range(2)]
                ctx_sb = acts.tile([128, 4 * GQ], BF)  # col = sc*GQ + h*DK + d
                ctxT = acts.tile([128, 2 * SBLK], BF)  # col = cj*SBLK + s_local

                for sb in range(NSB):
                    # scores^T + exp, loop p outer so ctx h=2p,2p+1 can start
                    # while the other p-half is still being exp'd.
                    for p in range(2):
                        for t in range(NT):
                            sc = ps_sc.tile([128, 2 * SBLK], F32, tag="sc")
                            for hl in range(2):
                                nc.tensor.matmul(
                                    sc[:, hl * SBLK : (hl + 1) * SBLK],
                                    KT[hl * 64 : (hl + 1) * 64, t * 128 : (t + 1) * 128],
                                    QT[hl * 64 : (hl + 1) * 64,
                                       p * S + sb * SBLK : p * S + (sb + 1) * SBLK],
                                )
                            dst = PT[p][:, t * 2 * SBLK : (t + 1) * 2 * SBLK]
                            if t in OFF_T:
                                nc.vector.tensor_scalar(
                                    dst.bitcast(I16),
                                    sc[:],
                                    EXP_A,
                                    EXP_B,
                                    mybir.AluOpType.mult,
                                    mybir.AluOpType.add,
                                )
                            else:
                                nc.scalar.activation(
                                    dst, sc[:], mybir.ActivationFunctionType.Exp,
                                    scale=0.125,
                                )

                    # Q for the next super-block fills the PE gap while
                    # ScalarE/DVE are still exp-ing this one.
                    if sb < 3:
                        for m in range(2):
                            q_proj(m, sb + 1)

                    # ctx per head; col 64 of each sc-group = softmax denom
                    for h in range(H):
                        p, hl = divmod(h, 2)
                        cps = ps_ctx.tile([128, 4 * (DK + 1)], F32, tag="c")
                        for sc_i in range(4):
                            for t in range(NT):
                                nc.tensor.matmul(
                                    cps[:, sc_i * 65 : sc_i * 65 + 65],
                                    PT[p][:, t * 2 * SBLK + hl * SBLK + sc_i * 128
                                         : t * 2 * SBLK + hl * SBLK + (sc_i + 1) * 128],
                                    Vb[:, t * 65 : (t + 1) * 65],
                                    start=(t == 0),
                                    stop=(t == NT - 1),
                                )
                        rc = sml.tile([128, 4], F32, tag="rc")
                        nc.vector.reciprocal(
                            rc[:], cps[:].rearrange("p (sc c) -> p sc c", c=65)[:, :, DK]
                        )
                        for sc_i in range(4):
                            nc.vector.tensor_scalar_mul(
                                ctx_sb[:, sc_i * GQ + h * DK : sc_i * GQ + (h + 1) * DK],
                                cps[:, sc_i * 65 : sc_i * 65 + DK],
                                rc[:, sc_i : sc_i + 1],
                            )

                    # ctxT via DMA transpose: ctxT[p, cj, s] = ctx_sb[s, cj*128+p]
                    ctxT3 = ctxT[:].rearrange("p (c s) -> p c s", c=2)
                    for sc_i in range(4):
                        nc.sync.dma_start_transpose(
                            ctxT3[:, :, sc_i * 128 : (sc_i + 1) * 128],
                            ctx_sb[:, sc_i * GQ : (sc_i + 1) * GQ],
                        )

                    # output projection (partial over this group's 256 dims)
                    for sc_i in range(4):
                        ot = outp.tile([128, DM], F32, tag="ot")
                        for nb in range(2):
                            ps = ps_pr.tile([128, SBLK], F32, tag="p")
                            for cj in range(2):
                                nc.tensor.matmul(
                                    ps[:],
                                    ctxT[:, cj * SBLK + sc_i * 128 : cj * SBLK + (sc_i + 1) * 128],
                                    wo_bf[:, cj * DM + nb * SBLK : cj * DM + (nb + 1) * SBLK],
                                    start=(cj == 0),
                                    stop=(cj == 1),
                                )
                            nc.vector.tensor_copy(ot[:, nb * SBLK : (nb + 1) * SBLK], ps[:])
                        row = sb * SBLK + sc_i * 128
                        nc.sync.dma_start(out[row : row + 128, :], ot[:])

            if iters == 1:
                _pipeline()
            else:
                with tc.For_i(0, iters):
                    _pipeline()

    _split_sync_waits(nc)
    return nc


def _make_in_maps(x, W_Q, b_Q, W_K, b_K, W_V, b_V, W_O):
    import ml_dtypes

    BF = ml_dtypes.bfloat16
    in_maps = []
    for c in range(8):
        b, g = divmod(c, 4)
        wq_g = W_Q[:, g * GQ : (g + 1) * GQ]  # [1024, 256]
        wq_l = np.ascontiguousarray(
            wq_g.reshape(NK, 128, GQ).transpose(1, 0, 2).reshape(128, NK * GQ)
        ).astype(BF)
        wkv_l = np.empty((128, NK, 128), np.float32)
        wkv_l[:, :, :DK] = W_K[g].reshape(NK, 128, DK).transpose(1, 0, 2)
        wkv_l[:, :, DK:] = W_V[g].reshape(NK, 128, DK).transpose(1, 0, 2)
        wkv_l = np.ascontiguousarray(wkv_l.reshape(128, NK * 128)).astype(BF)
        wo_g = W_O[g * GQ : (g + 1) * GQ, :]  # [256, 1024]
        wo_l = np.ascontiguousarray(
            wo_g.reshape(2, 128, DM).transpose(1, 0, 2).reshape(128, 2 * DM)
        ).astype(BF)
        bq_l = np.ascontiguousarray(b_Q[g * GQ : (g + 1) * GQ].reshape(2, 128).T)
        in_maps.append(
            {
                "x": np.ascontiguousarray(x[b]).astype(BF),
                "wq": wq_l,
                "wkv": wkv_l,
                "wo": wo_l,
                "bq": bq_l,
                "bk": np.ascontiguousarray(b_K[g].reshape(64, 1)),
            }
        )
    return in_maps


def kernel(x, W_Q, b_Q, W_K, b_K, W_V, b_V, W_O, b_O):
    from concourse.bass_utils import run_bass_kernel_spmd

    x = np.asarray(x, np.float32)
    W_Q, b_Q = np.asarray(W_Q, np.float32), np.asarray(b_Q, np.float32)
    W_K, b_K = np.asarray(W_K, np.float32), np.asarray(b_K, np.float32)
    W_V, b_V = np.asarray(W_V, np.float32), np.asarray(b_V, np.float32)
    W_O, b_O = np.asarray(W_O, np.float32), np.asarray(b_O, np.float32)

    if "nc" not in _CACHED:
        _CACHED["nc"] = _build_nc()
    nc = _CACHED["nc"]

    in_maps = _make_in_maps(x, W_Q, b_Q, W_K, b_K, W_V, b_V, W_O)
    res = run_bass_kernel_spmd(nc, in_maps, list(range(8)))

    out = np.zeros((B, S, DM), np.float32)
    for c in range(8):
        b, g = divmod(c, 4)
        out[b] += res.results[c]["out"]
    # host-side bias terms: b_O, plus b_V's exact contribution
    # (softmax rows sum to 1 -> ctx bias = tile(b_V[g]) per head)
    bv_full = np.concatenate([np.tile(b_V[g], H) for g in range(G)])  # [1024]
    out += (b_O + bv_full @ W_O)[None, None, :]
    return out


# revision 9
# speedup vs baseline: 1.1996x; 1.1996x over previous
"""Grouped-Query Attention on 8 Trainium2 NeuronCores.

Sharding: core c handles (batch b = c//4, query-head group g = c%4).
Each core computes its group's Q projection (256 cols of W_Q), the
group-shared K/V projections, 4 heads of attention over the full
sequence, and a partial output projection against the group's 256 rows
of W_O. The host sums the 4 group partials per batch and adds b_O.

v2 design (all matmuls bf16 operands, fp32 PSUM accumulate):
  - host pre-casts x and all weights to bf16 in SBUF-ready layouts
  - xT produced by hardware DMA-transpose (DRAM -> SBUF), no PE/DVE cost
  - K and V projections fused: stationary [128, 64K|64V] -> KT rows 0-63,
    VT rows 64-127 of one PSUM tile; V natural recovered from VT by a
    second DMA-transpose
  - softmax exp split between ScalarE (exact Exp activation) and DVE
    (Schraudolph bit-trick: PT_bits = int16(score * A + B), one
    tensor_scalar per offloaded chunk) to balance engine busy time
  - ctx matmuls accumulate [V | 1] so col 64 carries the softmax denom;
    evac divides by it (DVE reciprocal + scale-multiply)
  - ctxT produced by DMA-transpose; output projection accumulates over
    the two 128-row q-chunks; DVE evacuates PSUM -> staging -> DMA out

b_V and b_O are applied on the host: b_V adds exactly
(tile(b_V) @ W_O_g) to every output row (softmax weights sum to 1).
"""

import numpy as np

S = 2048
DM = 1024
G = 4
H = 4  # heads per group
DK = 64
GQ = 256  # query width per group
B = 2
NK = DM // 128  # 8 contraction chunks
NT = S // 128  # 16 token chunks
SBLK = 512
NSB = S // SBLK  # 4 query super-blocks

# Schraudolph constants: exp(s/8) = 2^(s*0.125*log2 e); bf16 bits via
# int16(s*A + B). B includes the -5.5 minimax offset and +0.5 for the
# truncating float->int convert on hardware.
EXP_A = float(np.float32(128.0 * 0.125 * np.log2(np.e)))
EXP_B = float(np.float32(127.0 * 128.0 - 5.5 + 0.5))
# t-chunks (of 16) whose exp goes to DVE instead of ScalarE
OFF_T = (1, 4, 7, 10, 13)

_CACHED = {}


def _split_sync_waits(nc, drain_max=1, other_max=1):
    """This walrus build has a single sync-wait slot on CTRL-class
    instructions (Drain/NoOp); Tile's exit drain collects 3+. Move the
    excess onto preceding single-wait NOPs on the same engine."""
    import concourse.mybir as mybir
    import bass_rust

    n_split = 0
    for f in nc.m.functions:
        for bb in f.blocks:
            out = []
            changed = False
            for inst in bb.instructions:
                si = getattr(inst, "sync_info", None)
                limit = drain_max if type(inst).__name__ in ("InstDrain", "InstNoOp") else other_max
                if si is not None and len(si.on_wait) > limit:
                    waits = list(si.on_wait)
                    keep = waits[-limit:] if limit else []
                    head = waits[: len(waits) - limit]
                    for w in head:
                        out.append(
                            mybir.InstNoOp(
                                name=f"{inst.name}-wsp{n_split}",
                                engine=inst.engine,
                                sync_info=mybir.SyncInfo(on_wait=[w], on_update=[]),
                                bass_nofuse=True,
                            )
                        )
                        n_split += 1
                    inst.sync_info = bass_rust.SyncInfo(on_wait=keep, on_update=si.on_update)
                    changed = True
                out.append(inst)
            if changed:
                bb.instructions = out
    return n_split


def _build_nc(iters=1):
    import concourse.bass as bass
    import concourse.mybir as mybir
    import concourse.tile as tile

    F32 = mybir.dt.float32
    BF = mybir.dt.bfloat16
    I16 = mybir.dt.int16

    nc = bass.Bass("TRN2", target_bir_lowering=False, debug=False, num_devices=8)
    x = nc.dram_tensor("x", [S, DM], BF, kind="ExternalInput")
    wq = nc.dram_tensor("wq", [128, NK * 128 * 2], BF, kind="ExternalInput")
    wkv = nc.dram_tensor("wkv", [128, NK * 128], BF, kind="ExternalInput")
    wo = nc.dram_tensor("wo", [128, 2 * DM], BF, kind="ExternalInput")
    bq = nc.dram_tensor("bq", [128, 2], F32, kind="ExternalInput")
    bk = nc.dram_tensor("bk", [64, 1], F32, kind="ExternalInput")
    out = nc.dram_tensor("out", [S, DM], F32, kind="ExternalOutput")

    with tile.TileContext(nc) as tc:
        with (
            tc.tile_pool(name="wts", bufs=1) as wts,
            tc.tile_pool(name="acts", bufs=1) as acts,
            tc.tile_pool(name="sml", bufs=2) as sml,
            tc.tile_pool(name="outp", bufs=4) as outp,
            tc.tile_pool(name="ps_sc", bufs=2, space="PSUM") as ps_sc,
            tc.tile_pool(name="ps_pr", bufs=2, space="PSUM") as ps_pr,
            tc.tile_pool(name="ps_ctx", bufs=2, space="PSUM") as ps_ctx,
        ):
            def _pipeline():
                # ---- xT via DMA transpose: xT[p, k, s] = x[s, k*128+p].
                # sg0 first so the K/V and Q projections can start while the
                # rest of x is still transposing; weights interleave after it.
                xT = acts.tile([128, NK * S], BF)
                xT3 = xT[:].rearrange("p (k s) -> p k s", k=NK)
                nc.sync.dma_start_transpose(xT3[:, :, :SBLK], x[:SBLK, :])

                kv_bf = wts.tile([128, NK * 128], BF)  # chunk k: cols [k*128,+64)=K, +64..128=V
                nc.sync.dma_start(kv_bf[:], wkv[:])
                wq_bf = wts.tile([128, NK * 256], BF)  # k-chunk k, m: cols k*256+m*128
                nc.sync.dma_start(wq_bf[:], wq[:])
                bq_t = wts.tile([128, 2], F32)
                nc.sync.dma_start(bq_t[:], bq[:])
                bk_t = wts.tile([64, 1], F32)
                nc.sync.dma_start(bk_t[:], bk[:])

                for sg in range(1, NSB):
                    nc.sync.dma_start_transpose(
                        xT3[:, :, sg * SBLK : (sg + 1) * SBLK],
                        x[sg * SBLK : (sg + 1) * SBLK, :],
                    )
                wo_bf = wts.tile([128, 2 * DM], BF)  # q-chunk cj at cols [cj*DM, ...)
                nc.sync.dma_start(wo_bf[:], wo[:])

                KT = acts.tile([128, S], BF)  # rows 64-127 duplicate 0-63
                VT = acts.tile([64, S], BF)
                Vb = acts.tile([128, NT * (DK + 1)], BF)  # [V | 1] per token chunk
                QT = acts.tile([128, 2 * S], BF)
                # PT per p-half so sb+1 scores reuse half a buffer while ctx of
                # heads 2,3 still reads the other.
                PT = [acts.tile([128, NT * 2 * SBLK], BF, name=f"pt{p}") for p in range(2)]
                ctx_sb = acts.tile([128, 4 * GQ], BF)  # col = sc*GQ + h*DK + d
                ctxT = acts.tile([128, 2 * SBLK], BF)  # col = cj*SBLK + s_local

                def kv_proj(sg):
                    ps = ps_pr.tile([128, SBLK], F32, tag="p")
                    for k in range(NK):
                        nc.tensor.matmul(
                            ps[:],
                            kv_bf[:, k * 128 : (k + 1) * 128],
                            xT[:, k * S + sg * SBLK : k * S + (sg + 1) * SBLK],
                            start=(k == 0),
                            stop=(k == NK - 1),
                        )
                    nc.vector.tensor_scalar_add(
                        KT[:64, sg * SBLK : (sg + 1) * SBLK], ps[:64, :], bk_t[:]
                    )
                    nc.vector.tensor_copy(VT[:, sg * SBLK : (sg + 1) * SBLK], ps[64:128, :])
                    # per-sg row duplication so early score chunks aren't
                    # gated on the full sequence
                    nc.sync.dma_start(
                        KT[64:128, sg * SBLK : (sg + 1) * SBLK],
                        KT[:64, sg * SBLK : (sg + 1) * SBLK],
                    )

                def q_proj(m, sg):
                    ps = ps_pr.tile([128, SBLK], F32, tag="p")
                    for k in range(NK):
                        nc.tensor.matmul(
                            ps[:],
                            wq_bf[:, k * 256 + m * 128 : k * 256 + (m + 1) * 128],
                            xT[:, k * S + sg * SBLK : k * S + (sg + 1) * SBLK],
                            start=(k == 0),
                            stop=(k == NK - 1),
                        )
                    nc.vector.tensor_scalar_add(
                        QT[:, m * S + sg * SBLK : m * S + (sg + 1) * SBLK], ps[:], bq_t[:, m : m + 1]
                    )

                def score_chunk(sb, p, t):
                    sc = ps_sc.tile([128, 2 * SBLK], F32, tag="sc")
                    for hl in range(2):
                        nc.tensor.matmul(
                            sc[:, hl * SBLK : (hl + 1) * SBLK],
                            KT[hl * 64 : (hl + 1) * 64, t * 128 : (t + 1) * 128],
                            QT[hl * 64 : (hl + 1) * 64,
                               p * S + sb * SBLK : p * S + (sb + 1) * SBLK],
                        )
                    dst = PT[p][:, t * 2 * SBLK : (t + 1) * 2 * SBLK]
                    if t in OFF_T:
                        nc.vector.tensor_scalar(
                            dst.bitcast(I16), sc[:], EXP_A, EXP_B,
                            mybir.AluOpType.mult, mybir.AluOpType.add,
                        )
                    else:
                        nc.scalar.activation(
                            dst, sc[:], mybir.ActivationFunctionType.Exp, scale=0.125
                        )

                def ctx_accum(cps, h, t):
                    # One accumulation group for the whole head: a start=True
                    # per sc_i would mark the shared 2KB PSUM zero-region while
                    # sibling groups are mid-flight and wipe their partials.
                    # With a single start, each 65-col slice is lazily zeroed
                    # at its first write.
                    p, hl = divmod(h, 2)
                    for sc_i in range(4):
                        nc.tensor.matmul(
                            cps[:, sc_i * 65 : sc_i * 65 + 65],
                            PT[p][:, t * 2 * SBLK + hl * SBLK + sc_i * 128
                                 : t * 2 * SBLK + hl * SBLK + (sc_i + 1) * 128],
                            Vb[:, t * 65 : (t + 1) * 65],
                            start=(t == 0 and sc_i == 0),
                            stop=(t == NT - 1 and sc_i == 3),
                        )

                def ctx_evac(cps, h):
                    rc = sml.tile([128, 4], F32, tag="rc")
                    nc.vector.reciprocal(
                        rc[:], cps[:].rearrange("p (sc c) -> p sc c", c=65)[:, :, DK]
                    )
                    for sc_i in range(4):
                        nc.vector.tensor_scalar_mul(
                            ctx_sb[:, sc_i * GQ + h * DK : sc_i * GQ + (h + 1) * DK],
                            cps[:, sc_i * 65 : sc_i * 65 + DK],
                            rc[:, sc_i : sc_i + 1],
                        )

                def ctx_head(sb, h):
                    cps = ps_ctx.tile([128, 4 * (DK + 1)], F32, tag="c")
                    for t in range(NT):
                        ctx_accum(cps, h, t)
                    ctx_evac(cps, h)

                def ctxT_half(cj):
                    ctxT3 = ctxT[:].rearrange("p (c s) -> p c s", c=2)
                    for sc_i in range(4):
                        nc.sync.dma_start_transpose(
                            ctxT3[:, cj : cj + 1, sc_i * 128 : (sc_i + 1) * 128],
                            ctx_sb[:, sc_i * GQ + cj * 128 : sc_i * GQ + (cj + 1) * 128],
                        )

                def out_group(sb, sc_i):
                    ot = outp.tile([128, DM], F32, tag="ot")
                    for nb in range(2):
                        ps = ps_pr.tile([128, SBLK], F32, tag="p")
                        for cj in range(2):
                            nc.tensor.matmul(
                                ps[:],
                                ctxT[:, cj * SBLK + sc_i * 128 : cj * SBLK + (sc_i + 1) * 128],
                                wo_bf[:, cj * DM + nb * SBLK : cj * DM + (nb + 1) * SBLK],
                                start=(cj == 0),
                                stop=(cj == 1),
                            )
                        nc.vector.tensor_copy(ot[:, nb * SBLK : (nb + 1) * SBLK], ps[:])
                    row = sb * SBLK + sc_i * 128
                    nc.sync.dma_start(out[row : row + 128, :], ot[:])

                # ---- preamble: Q/KV interleaved with early sb0-p0 scores ----
                kv_proj(0)
                for m in range(2):
                    q_proj(m, 0)
                for t in range(4):
                    score_chunk(0, 0, t)
                kv_proj(1)
                for t in range(4, 8):
                    score_chunk(0, 0, t)
                kv_proj(2)
                for t in range(8, 12):
                    score_chunk(0, 0, t)
                kv_proj(3)
                for t in range(12, 16):
                    score_chunk(0, 0, t)

                # V natural from VT by DMA transpose (needs contiguous dst;
                # sliced transpose dsts mis-execute on hardware).
                Vtmp = acts.tile([128, NT * DK], BF)
                nc.sync.dma_start_transpose(
                    Vtmp[:].rearrange("p (t c) -> p t c", c=DK), VT[:]
                )
                nc.vector.tensor_copy(
                    Vb[:].rearrange("p (t c) -> p t c", c=DK + 1)[:, :, :DK],
                    Vtmp[:].rearrange("p (t c) -> p t c", c=DK),
                )
                nc.gpsimd.memset(Vb[:].rearrange("p (t c) -> p t c", c=DK + 1)[:, :, DK], 1.0)

                # sb0 phase 2: scores p1 interleaved with Q(sg1) and ctx h0/h1
                for t in range(8):
                    score_chunk(0, 1, t)
                q_proj(0, 1)
                for t in range(8, 12):
                    score_chunk(0, 1, t)
                q_proj(1, 1)
                for t in range(12, 16):
                    score_chunk(0, 1, t)
                ctx_head(0, 0)
                ctx_head(0, 1)
                ctxT_half(0)

                # ---- software-pipelined steady state ----
                for sb in range(1, NSB):
                    # phase 1: scores p0(sb) interleaved with ctx h2/h3 +
                    # ctxT cj1 + out of sb-1
                    for t in range(4):
                        score_chunk(sb, 0, t)
                    ctx_head(sb - 1, 2)
                    for t in range(4, 8):
                        score_chunk(sb, 0, t)
                    ctx_head(sb - 1, 3)
                    ctxT_half(1)
                    for t in range(8, 12):
                        score_chunk(sb, 0, t)
                    out_group(sb - 1, 0)
                    out_group(sb - 1, 1)
                    for t in range(12, 16):
                        score_chunk(sb, 0, t)
                    out_group(sb - 1, 2)
                    out_group(sb - 1, 3)
                    # phase 2: scores p1(sb) interleaved with Q(sb+1) and
                    # ctx h0/h1(sb). For sb3, all four heads' ctx accumulation
                    # is spread through the phase (h0/h1 first half against the
                    # completed p0 exps, then h2/h3 chasing the p1 exp stream)
                    # so the tail collapses to evacs + out.
                    if sb < 3:
                        for t in range(4):
                            score_chunk(sb, 1, t)
                        q_proj(0, sb + 1)
                        for t in range(4, 8):
                            score_chunk(sb, 1, t)
                        q_proj(1, sb + 1)
                        for t in range(8, 12):
                            score_chunk(sb, 1, t)
                        ctx_head(sb, 0)
                        for t in range(12, 16):
                            score_chunk(sb, 1, t)
                        ctx_head(sb, 1)
                        ctxT_half(0)

                # ---- sb3 phase 2 + epilogue ----
                cps0 = ps_ctx.tile([128, 4 * (DK + 1)], F32, tag="c", name="cc0")
                cps1 = ps_ctx.tile([128, 4 * (DK + 1)], F32, tag="c", name="cc1")
                for t in range(8):
                    score_chunk(3, 1, t)
                    ctx_accum(cps0, 0, 2 * t)
                    ctx_accum(cps0, 0, 2 * t + 1)
                    ctx_accum(cps1, 1, 2 * t)
                    ctx_accum(cps1, 1, 2 * t + 1)
                ctx_evac(cps0, 0)
                ctx_evac(cps1, 1)
                ctxT_half(0)
                cps2 = ps_ctx.tile([128, 4 * (DK + 1)], F32, tag="c", name="cc2")
                cps3 = ps_ctx.tile([128, 4 * (DK + 1)], F32, tag="c", name="cc3")
                for t in range(8, 16):
                    score_chunk(3, 1, t)
                    ctx_accum(cps2, 2, 2 * (t - 8))
                    ctx_accum(cps2, 2, 2 * (t - 8) + 1)
                    ctx_accum(cps3, 3, 2 * (t - 8))
                    ctx_accum(cps3, 3, 2 * (t - 8) + 1)
                ctx_evac(cps2, 2)
                ctx_evac(cps3, 3)
                ctxT_half(1)
                for sc_i in range(4):
                    out_group(3, sc_i)

            if iters == 1:
                _pipeline()
            else:
                with tc.For_i(0, iters):
                    _pipeline()

    _split_sync_waits(nc)
    return nc


def _make_in_maps(x, W_Q, b_Q, W_K, b_K, W_V, b_V, W_O):
    import ml_dtypes

    BF = ml_dtypes.bfloat16
    in_maps = []
    for c in range(8):
        b, g = divmod(c, 4)
        wq_g = W_Q[:, g * GQ : (g + 1) * GQ]  # [1024, 256]
        wq_l = np.ascontiguousarray(
            wq_g.reshape(NK, 128, GQ).transpose(1, 0, 2).reshape(128, NK * GQ)
        ).astype(BF)
        wkv_l = np.empty((128, NK, 128), np.float32)
        wkv_l[:, :, :DK] = W_K[g].reshape(NK, 128, DK).transpose(1, 0, 2)
        wkv_l[:, :, DK:] = W_V[g].reshape(NK, 128, DK).transpose(1, 0, 2)
        wkv_l = np.ascontiguousarray(wkv_l.reshape(128, NK * 128)).astype(BF)
        wo_g = W_O[g * GQ : (g + 1) * GQ, :]  # [256, 1024]
        wo_l = np.ascontiguousarray(
            wo_g.reshape(2, 128, DM).transpose(1, 0, 2).reshape(128, 2 * DM)
        ).astype(BF)
        bq_l = np.ascontiguousarray(b_Q[g * GQ : (g + 1) * GQ].reshape(2, 128).T)
        in_maps.append(
            {
                "x": np.ascontiguousarray(x[b]).astype(BF),
                "wq": wq_l,
                "wkv": wkv_l,
                "wo": wo_l,
                "bq": bq_l,
                "bk": np.ascontiguousarray(b_K[g].reshape(64, 1)),
            }
        )
    return in_maps


def kernel(x, W_Q, b_Q, W_K, b_K, W_V, b_V, W_O, b_O):
    from concourse.bass_utils import run_bass_kernel_spmd

    x = np.asarray(x, np.float32)
    W_Q, b_Q = np.asarray(W_Q, np.float32), np.asarray(b_Q, np.float32)
    W_K, b_K = np.asarray(W_K, np.float32), np.asarray(b_K, np.float32)
    W_V, b_V = np.asarray(W_V, np.float32), np.asarray(b_V, np.float32)
    W_O, b_O = np.asarray(W_O, np.float32), np.asarray(b_O, np.float32)

    if "nc" not in _CACHED:
        _CACHED["nc"] = _build_nc()
    nc = _CACHED["nc"]

    in_maps = _make_in_maps(x, W_Q, b_Q, W_K, b_K, W_V, b_V, W_O)
    res = run_bass_kernel_spmd(nc, in_maps, list(range(8)))

    out = np.zeros((B, S, DM), np.float32)
    for c in range(8):
        b, g = divmod(c, 4)
        out[b] += res.results[c]["out"]
    # host-side bias terms: b_O, plus b_V's exact contribution
    # (softmax rows sum to 1 -> ctx bias = tile(b_V[g]) per head)
    bv_full = np.concatenate([np.tile(b_V[g], H) for g in range(G)])  # [1024]
    out += (b_O + bv_full @ W_O)[None, None, :]
    return out
